# revision 27
# baseline (speedup 1.0000x reference)
"""Multi-head attention with q/v LoRA on 8 trn2 NeuronCores (bf16 PE path).

Reference computation (B=2, N=2048, C=1024, H=16, HD=64, R=16):
    qkv = x @ w_qkv + b_qkv                -> split per-head q, k, v
    q  += ((q @ a_q) @ b_q) * 2.0          (per head; same for v)
    out = softmax(q k^T / 8) v             (full N x N scores)
    y   = out @ w_proj + b_proj

Sharding: tensor-parallel over heads -- each of the 8 cores owns 2 heads
(128 of the 3*1024 qkv columns) for both batches; attention output is
resharded over tokens with a per-(batch,head) AllToAll so each core
computes final proj rows for its 256 tokens per batch with the full
w_proj.

Key implementation choices:
  * All PE operands are bf16 (hardware runs fp32r at ~2 cycles/row; bf16
    at 1).  PSUM accumulation stays fp32, biases stay fp32.
  * The LoRA is linear in q/v, so it is folded into the qkv weights on
    the host: w_eff = w @ (I + blockdiag(a@b)*scale), same for bias.
    Nothing LoRA-related runs on device.
  * x^T, weights are pre-cast to bf16 on the host and DMA'd straight
    into matmul operand tiles (no on-device rounding copies).
  * Softmax: scores S^T = k^T' q^T -> exp on ACT (bf16 out) -> P @ [v|1]
    in PSUM (ones column gives the row sums).  1/sums via the fast DVE
    reciprocal, broadcast to 64 partitions on the idle GpSimd engine,
    one fused multiply writes normalized bf16 O^T for the AllToAll.
  * v_aug ones columns are memset once into two persistent buffers.
  * AllToAll payloads are bf16 (256 KB per (batch, head)); the receive
    side DMAs the collective output straight into the proj operand tile.
The host stitches the 8 token shards and transposes back to [B, N, C].
"""

import sys

sys.path.insert(0, "/opt/trn_rl_repo")
sys.path.insert(0, "/root/.axon_site")

import numpy as np
import ml_dtypes

import concourse.bass as bass
import concourse.mybir as mybir
import concourse.tile as tile
from concourse.bass_utils import run_bass_kernel_spmd

f32 = mybir.dt.float32
bf16 = mybir.dt.bfloat16
AF = mybir.ActivationFunctionType

B, N, C = 2, 2048, 1024
H, HD, R = 16, 64, 16
LORA_SCALE = 32.0 / R
ATTN_SCALE = HD ** -0.5
NCORES = 8
HPC = H // NCORES          # heads per core = 2
PC = HPC * HD              # partition columns per core = 128
ROWS = B * N               # 4096 tokens
RC = 512                   # row-chunk size for qkv production
TPC = N // NCORES          # tokens per core per batch = 256


def _legalize_waits(nc, max_waits=1):
    """This walrus build accepts at most one sync-wait per instruction;
    Tile attaches several.  Move surplus waits onto same-engine NoOps
    inserted immediately before the instruction (identical semantics)."""
    counter = 0
    for fn in nc.m.functions:
        for bb in fn.blocks:
            insts = bb.instructions
            out = []
            changed = False
            for inst in insts:
                si = inst.sync_info
                if si is not None and si.on_wait and len(si.on_wait) > max_waits:
                    waits = list(si.on_wait)
                    for w in waits[:-max_waits]:
                        counter += 1
                        nop = mybir.InstNoOp(
                            name=f"I-wfix-{counter}",
                            engine=inst.engine,
                            sync_info=mybir.SyncInfo(on_wait=[w], on_update=[]),
                        )
                        nc.register_instruction(nop)
                        out.append(nop)
                    si.on_wait.clear()
                    si.on_wait.extend(waits[-max_waits:])
                    changed = True
                out.append(inst)
            if changed:
                insts[:] = out


def build_nc():
    nc = bass.Bass(num_devices=NCORES)

    xt_d = nc.dram_tensor("xt", [C, ROWS], bf16, kind="ExternalInput")
    wq_d = nc.dram_tensor("wq", [128, 1024], bf16, kind="ExternalInput")
    wk_d = nc.dram_tensor("wk", [128, 1024], bf16, kind="ExternalInput")
    wv_d = nc.dram_tensor("wv", [128, 1024], bf16, kind="ExternalInput")
    bq_d = nc.dram_tensor("bq", [128, 1], f32, kind="ExternalInput")
    bk_d = nc.dram_tensor("bk", [128, 1], f32, kind="ExternalInput")
    bv_d = nc.dram_tensor("bv", [128, 1], f32, kind="ExternalInput")
    wp_d = nc.dram_tensor("wp", [128, 8 * 1024], bf16, kind="ExternalInput")
    bp_d = nc.dram_tensor("bp", [128, 8], f32, kind="ExternalInput")
    eye64x2_d = nc.dram_tensor("eye64x2", [128, 64], bf16, kind="ExternalInput")
    out_d = nc.dram_tensor("out", [B, C, TPC], f32, kind="ExternalOutput")

    with nc.allow_low_precision(
        reason="bf16 matmul operands are intended; PSUM accumulation stays fp32"
    ), tile.TileContext(nc) as tc:
        with (
            tc.tile_pool(name="persist", bufs=1) as persist,
            tc.tile_pool(name="const", bufs=1) as const,
            tc.tile_pool(name="dram", bufs=1, space="DRAM") as dram,
            tc.tile_pool(name="xio", bufs=2) as xio_p,
            tc.tile_pool(name="work", bufs=2) as work_p,
            tc.tile_pool(name="ps", bufs=1, space="PSUM") as ps,
        ):
            qT = persist.tile([128, ROWS], bf16, tag="qT", name="qT")
            kT = persist.tile([128, ROWS], bf16, tag="kT", name="kT")
            vT = persist.tile([128, ROWS], bf16, tag="vT", name="vT")

            # prefetch the first x^T chunk ahead of the weight DMAs
            xT0 = xio_p.tile([128, 8 * RC], bf16, tag="xT", bufs=6, name="xT00")
            nc.sync.dma_start(
                out=xT0[:].rearrange("p (a r) -> p a r", a=8),
                in_=xt_d[:, 0:RC].rearrange("(a p) r -> p a r", p=128),
            )

            w_t = []
            for nm, d in (("wq", wq_d), ("wk", wk_d), ("wv", wv_d)):
                t = const.tile([128, 1024], bf16, tag=nm, name=f"{nm}_t")
                nc.sync.dma_start(out=t[:], in_=d[:])
                w_t.append(t)

            eye64x2 = const.tile([128, 64], bf16, tag="eye64", name="eye64")
            nc.sync.dma_start(out=eye64x2[:], in_=eye64x2_d[:])

            bias_t = []
            for nm, d in (("bq", bq_d), ("bk", bk_d), ("bv", bv_d)):
                bt = const.tile([128, 1], f32, tag=nm, name=f"{nm}_t")
                nc.sync.dma_start(out=bt[:], in_=d[:])
                bias_t.append(bt)
            bp_t = const.tile([128, 8], f32, tag="bp", name="bp_t")
            nc.sync.dma_start(out=bp_t[:], in_=bp_d[:])

            wp_t = const.tile([128, 8 * 1024], bf16, tag="wp_t", name="wp_t")

            # persistent v_aug buffers (one per (batch, head)): ones columns
            # written once by memset, data blocks overwritten by transposes
            v_aug_bufs = {}
            for b in range(B):
                for hl in range(HPC):
                    va = persist.tile(
                        [128, 16 * 65], bf16, tag=f"va{b}{hl}", name=f"va{b}{hl}"
                    )
                    nc.gpsimd.memset(va[:], 1.0)
                    v_aug_bufs[(b, hl)] = va

            ones_row = const.tile([1, 64], bf16, tag="ones_r", name="ones_r")
            nc.gpsimd.memset(ones_row[:], 1.0)

            qkvT = (qT, kT, vT)

            xts = {}

            def load_xchunk(b, rci):
                r0 = b * N + rci * RC
                xT_t = xio_p.tile(
                    [128, 8 * RC], bf16, tag="xT", bufs=6, name=f"xT{b}{rci}"
                )
                nc.sync.dma_start(
                    out=xT_t[:].rearrange("p (a r) -> p a r", a=8),
                    in_=xt_d[:, r0 : r0 + RC].rearrange("(a p) r -> p a r", p=128),
                )
                xts[(b, rci)] = xT_t
                return xT_t

            def emit_qkv_m(b, rci, m, act_ok=True):
                r0 = b * N + rci * RC
                xT_t = xts[(b, rci)]
                acc = ps.tile([128, RC], f32, tag="acc", bufs=2, name=f"ac{b}{rci}{m}")
                for ci in range(8):
                    nc.tensor.matmul(
                        acc[:],
                        w_t[m][:, ci * 128 : (ci + 1) * 128],
                        xT_t[:, ci * RC : (ci + 1) * RC],
                        start=(ci == 0),
                        stop=(ci == 7),
                    )
                dst = qkvT[m][:, r0 : r0 + RC]
                if m == 0 and act_ok:
                    nc.scalar.activation(dst, acc[:], AF.Identity, bias=bias_t[m][:])
                else:
                    nc.vector.tensor_scalar_add(dst, acc[:], bias_t[m][:])

            def emit_qkv_chunk(b, rci, xT_t=None, act_ok=True):
                if (b, rci) not in xts:
                    if xT_t is not None:
                        xts[(b, rci)] = xT_t
                    else:
                        load_xchunk(b, rci)
                for m in range(3):
                    emit_qkv_m(b, rci, m, act_ok=act_ok)

            def emit_vaug(b, hl):
                boff = b * N
                hs = slice(hl * HD, (hl + 1) * HD)
                v_aug = v_aug_bufs[(b, hl)]
                for kt4 in range(4):
                    vtr = ps.tile([128, 256], bf16, tag="s", bufs=2, name=f"vt{b}{hl}{kt4}")
                    for j in range(4):
                        ko = boff + (kt4 * 4 + j) * 128
                        nc.tensor.transpose(
                            vtr[:, j * 64 : (j + 1) * 64],
                            vT[hs, ko : ko + 128],
                            eye64x2[hs, :],
                        )
                    nc.vector.tensor_copy(
                        v_aug[:].rearrange("p (k e) -> p k e", e=65)[
                            :, kt4 * 4 : kt4 * 4 + 4, 0:64
                        ],
                        vtr[:].rearrange("p (k e) -> p k e", e=64),
                    )
                return v_aug

            # filler queue: small batches of independent PE work emitted
            # between attention kt iterations so the tensor engine never
            # starves while waiting on the ACT exp cadence
            filler_q = []

            def pop_filler():
                if filler_q:
                    filler_q.pop(0)()

            def emit_unit(b, hl, qh, a2a_in, pops=(4, 8, 12)):
                """Emit scores/exp/PV for one (batch, head, q-half) unit.
                Returns a finisher closure (normalize + a2a staging DMA) to
                be emitted later -- after the next unit's first matmuls -- so
                the slow reciprocal never blocks the in-order PE queue."""
                boff = b * N
                hs = slice(hl * HD, (hl + 1) * HD)
                qoff = boff + qh * 1024
                v_aug = v_aug_bufs[(b, hl)]
                o_ps = ps.tile([65, 1024], f32, tag="o", bufs=1, name=f"o{b}{hl}{qh}")

                def emit_pv(p_tile, kt):
                    for qc in range(2):
                        nc.tensor.matmul(
                            o_ps[:, qc * 512 : (qc + 1) * 512],
                            v_aug[:, kt * 65 : kt * 65 + 65],
                            p_tile[:, qc * 512 : (qc + 1) * 512],
                            start=(kt == 0),
                            stop=(kt == 15),
                        )

                pending = None
                for kt in range(16):
                    ko = boff + kt * 128
                    s_ps = ps.tile([128, 1024], f32, tag="s", bufs=2, name=f"s{b}{hl}{qh}{kt}")
                    for qc in range(2):
                        nc.tensor.matmul(
                            s_ps[:, qc * 512 : (qc + 1) * 512],
                            kT[hs, ko : ko + 128],
                            qT[hs, qoff + qc * 512 : qoff + (qc + 1) * 512],
                            start=True,
                            stop=True,
                        )
                    p_sb = work_p.tile([128, 1024], bf16, tag="p", bufs=3, name=f"p{qh}{kt}")
                    nc.scalar.activation(p_sb[:], s_ps[:], AF.Exp, scale=ATTN_SCALE)
                    if pending is not None:
                        emit_pv(*pending)
                        if kt in pops:
                            pop_filler()
                    pending = (p_sb, kt)
                emit_pv(*pending)
                # DVE-side epilogue now (doesn't touch the PE queue): copy
                # O^T + sums out of PSUM, take reciprocals per q-half
                nst = work_p.tile([65, 1024], f32, tag="nst", bufs=2, name=f"n{hl}{qh}")
                nc.vector.tensor_copy(nst[:], o_ps[:])
                r_bf = work_p.tile([1, 1024], bf16, tag="rbf", bufs=2, name=f"rb{b}{hl}{qh}")
                for rq in range(4):
                    r_sb = work_p.tile([1, 256], f32, tag="r", bufs=2, name=f"r{b}{hl}{qh}{rq}")
                    nc.vector.reciprocal(r_sb[:], nst[64:65, rq * 256 : (rq + 1) * 256])
                    nc.vector.tensor_copy(r_bf[:, rq * 256 : (rq + 1) * 256], r_sb[:])

                def finisher():
                    onrm = work_p.tile(
                        [64, 1024], bf16, tag="onrm", bufs=2, name=f"on{b}{hl}{qh}"
                    )
                    for qc in range(2):
                        bc_ps = ps.tile([64, 512], f32, tag="acc", bufs=2, name=f"bc{qc}")
                        nc.tensor.matmul(
                            bc_ps[:],
                            ones_row[:],
                            r_bf[:, qc * 512 : (qc + 1) * 512],
                            start=True,
                            stop=True,
                        )
                        nc.vector.tensor_mul(
                            onrm[:, qc * 512 : (qc + 1) * 512],
                            nst[0:64, qc * 512 : (qc + 1) * 512],
                            bc_ps[:],
                        )
                    nc.sync.dma_start(
                        out=a2a_in[qh * 4 : (qh + 1) * 4, :, :].rearrange(
                            "a p r -> p a r"
                        ),
                        in_=onrm[:].rearrange("p (a r) -> p a r", a=4),
                    )

                return finisher

            def emit_a2a(b, hl, a2a_in):
                a2a_out = dram.tile(
                    [8, 64, TPC], bf16, tag=f"ao{b}{hl}", name=f"ao{b}{hl}"
                )
                nc.gpsimd.collective_compute(
                    "AllToAll",
                    mybir.AluOpType.bypass,
                    replica_groups=[list(range(NCORES))],
                    ins=[a2a_in[:].opt()],
                    outs=[a2a_out[:].opt()],
                )
                return a2a_out

            def new_a2a_in(b, hl):
                return dram.tile([8, 64, TPC], bf16, tag=f"ai{b}{hl}", name=f"ai{b}{hl}")

            recv_tiles = {}

            def get_recv(b):
                if b not in recv_tiles:
                    recv_tiles[b] = work_p.tile(
                        [128, 8 * TPC], bf16, tag=f"rcr{b}", bufs=1, name=f"rr{b}"
                    )
                return recv_tiles[b]

            def emit_recv_head(b, hl, a2a_out):
                recv_r = get_recv(b)
                nc.sync.dma_start(
                    out=recv_r[hl * 64 : (hl + 1) * 64, :].rearrange(
                        "p (a r) -> p a r", a=8
                    ),
                    in_=a2a_out[:].rearrange("a p r -> p a r"),
                )
                return recv_r

            def emit_proj_mt(b, recv_r, mt):
                y_ps = ps.tile([128, TPC], f32, tag="acc", bufs=2, name=f"y{b}{mt}")
                for kc in range(8):
                    nc.tensor.matmul(
                        y_ps[:],
                        wp_t[:, kc * 1024 + mt * 128 : kc * 1024 + (mt + 1) * 128],
                        recv_r[:, kc * TPC : (kc + 1) * TPC],
                        start=(kc == 0),
                        stop=(kc == 7),
                    )
                yst = work_p.tile([128, TPC], f32, tag="yst", bufs=3, name=f"ys{b}{mt}")
                nc.vector.tensor_scalar_add(yst[:], y_ps[:], bp_t[:, mt : mt + 1])
                nc.sync.dma_start(
                    out=out_d[b, mt * 128 : (mt + 1) * 128, :], in_=yst[:]
                )

            def emit_proj_pass(b, recv_r, mt, y_ap, hl, last):
                """K=64 proj pass over one head-pair block; accumulates into
                y_ap across the two passes (hl=0 start, hl=1 stop)."""
                lo, hi = hl * 64, (hl + 1) * 64
                for kc in range(8):
                    nc.tensor.matmul(
                        y_ap,
                        wp_t[lo:hi, kc * 1024 + mt * 128 : kc * 1024 + (mt + 1) * 128],
                        recv_r[lo:hi, kc * TPC : (kc + 1) * TPC],
                        start=(hl == 0 and kc == 0),
                        stop=(last and kc == 7),
                        skip_group_check=True,
                    )
                if last:
                    yst = work_p.tile(
                        [128, TPC], f32, tag="yst", bufs=3, name=f"ys{b}{mt}"
                    )
                    nc.vector.tensor_scalar_add(yst[:], y_ap, bp_t[:, mt : mt + 1])
                    nc.sync.dma_start(
                        out=out_d[b, mt * 128 : (mt + 1) * 128, :], in_=yst[:]
                    )

            # ---- emission schedule ---------------------------------------
            # Phase 1: qkv(b0) back-to-back (dense PE stream ramps the
            # clock) + b0 v_aug transposes; b1 x-chunk DMAs pre-issued.
            emit_qkv_chunk(0, 0, xT_t=xT0)
            for rci in range(1, 4):
                emit_qkv_chunk(0, rci)
            for rci in range(4):
                load_xchunk(1, rci)
            nc.sync.dma_start(out=wp_t[:], in_=wp_d[:])
            emit_vaug(0, 0)
            emit_vaug(0, 1)

            # Phase 2: attention units.  qkv(b1), v_aug(b1), normalize
            # finishers, a2a/recv issues and proj blocks all run as filler
            # between kt iterations so the tensor engine never idles (idle
            # gaps drop the PE out of its boosted clock state).
            ai = {
                (b, hl): new_a2a_in(b, hl) for b in range(B) for hl in range(HPC)
            }
            ao = {}
            recv0 = get_recv(0)
            recv1 = get_recv(1)

            for rci in range(4):
                for m in range(3):
                    filler_q.append(
                        lambda rci=rci, m=m: emit_qkv_m(1, rci, m, act_ok=False)
                    )
            filler_q.append(lambda: emit_vaug(1, 0))
            filler_q.append(lambda: emit_vaug(1, 1))

            fin = emit_unit(0, 0, 0, ai[(0, 0)])
            filler_q.insert(0, fin)
            fin = emit_unit(0, 0, 1, ai[(0, 0)])
            filler_q.insert(0, fin)
            filler_q.insert(1, lambda: ao.update({(0, 0): emit_a2a(0, 0, ai[(0, 0)])}))
            fin = emit_unit(0, 1, 0, ai[(0, 1)])
            filler_q.insert(0, fin)
            fin = emit_unit(0, 1, 1, ai[(0, 1)])
            filler_q.insert(0, fin)
            filler_q.insert(1, lambda: ao.update({(0, 1): emit_a2a(0, 1, ai[(0, 1)])}))
            filler_q.insert(2, lambda: emit_recv_head(0, 0, ao[(0, 0)]))
            # b1 attention consumes qkv(b1)/v_aug(b1): drain any fillers
            # that haven't been popped yet before the first b1 unit
            while filler_q:
                pop_filler()
            fin = emit_unit(1, 0, 0, ai[(1, 0)])
            filler_q.insert(0, fin)
            filler_q.insert(1, lambda: emit_recv_head(0, 1, ao[(0, 1)]))
            for mt in range(4):
                filler_q.append(lambda mt=mt: emit_proj_mt(0, recv0, mt))
            fin = emit_unit(1, 0, 1, ai[(1, 0)])
            filler_q.insert(0, fin)
            filler_q.insert(1, lambda: ao.update({(1, 0): emit_a2a(1, 0, ai[(1, 0)])}))
            fin = emit_unit(1, 1, 0, ai[(1, 1)], pops=(2, 4, 6, 8, 10, 12, 14))
            filler_q.insert(0, fin)
            filler_q.insert(1, lambda: emit_recv_head(1, 0, ao[(1, 0)]))
            for mt in range(4, 8):
                filler_q.append(lambda mt=mt: emit_proj_mt(0, recv0, mt))
            fin = emit_unit(1, 1, 1, ai[(1, 1)], pops=(2, 4, 6, 8, 10, 12, 14))
            while filler_q:
                pop_filler()
            fin()
            ao[(1, 1)] = emit_a2a(1, 1, ai[(1, 1)])

            # Phase 3: proj(1) in two K=64 passes -- pass A (head block 0)
            # overlaps the in-flight a2a(1,1); pass B after its receive.
            # PSUM matmul start=True resets the whole 2 KB bank, so every mt
            # accumulator must own a distinct bank: spread the 8 groups over
            # the (now idle) s/o/acc pool slots, one group per bank.
            y1a = ps.tile([128, 1024], f32, tag="s", bufs=2, name="y1a")
            y1b = ps.tile([128, 1024], f32, tag="s", bufs=2, name="y1b")
            y1c = ps.tile([128, 1024], f32, tag="o", bufs=1, name="y1c")
            y1d = ps.tile([128, 512], f32, tag="acc", bufs=2, name="y1d")
            y1e = ps.tile([128, 512], f32, tag="acc", bufs=2, name="y1e")

            def y_ap(mt):
                # mts 0-5: two per two-bank tile, one per bank (cols 0-255
                # in bank 0, cols 512-767 in bank 1); mts 6-7: one-bank tiles
                if mt < 6:
                    t = (y1a, y1b, y1c)[mt // 2]
                    return t[:, (mt % 2) * 512 : (mt % 2) * 512 + TPC]
                t = y1d if mt == 6 else y1e
                return t[:, 0:TPC]

            for mt in range(8):
                emit_proj_pass(1, recv1, mt, y_ap(mt), 0, last=False)
            emit_recv_head(1, 1, ao[(1, 1)])
            for mt in range(8):
                emit_proj_pass(1, recv1, mt, y_ap(mt), 1, last=True)

    _legalize_waits(nc)
    return nc


_NC_CACHE = None


def _get_nc():
    global _NC_CACHE
    if _NC_CACHE is None:
        _NC_CACHE = build_nc()
    return _NC_CACHE


def _make_in_maps(inputs):
    bf = ml_dtypes.bfloat16
    x = np.ascontiguousarray(np.asarray(inputs["x"], dtype=np.float32)).reshape(ROWS, C)
    xt = np.ascontiguousarray(x.T.astype(bf))   # [C, ROWS] bf16
    w_qkv = np.asarray(inputs["w_qkv"], dtype=np.float32)
    b_qkv = np.asarray(inputs["b_qkv"], dtype=np.float32)
    a_q = np.asarray(inputs["a_q"], dtype=np.float32)
    b_q = np.asarray(inputs["b_q"], dtype=np.float32)
    a_v = np.asarray(inputs["a_v"], dtype=np.float32)
    b_v = np.asarray(inputs["b_v"], dtype=np.float32)
    w_proj = np.asarray(inputs["w_proj"], dtype=np.float32)
    b_proj = np.asarray(inputs["b_proj"], dtype=np.float32)

    # fold the (linear) per-head LoRA into the q/v weights and biases:
    # q_final = (x@w_q + b_q) @ (I + a_q@b_q * scale)
    dq = a_q @ b_q * LORA_SCALE                 # [64, 64]
    dv = a_v @ b_v * LORA_SCALE
    mq = np.eye(128, dtype=np.float32)
    mq[0:64, 0:64] += dq
    mq[64:128, 64:128] += dq
    mv = np.eye(128, dtype=np.float32)
    mv[0:64, 0:64] += dv
    mv[64:128, 64:128] += dv

    eye64x2 = np.vstack([np.eye(64, dtype=np.float32)] * 2).astype(bf)

    def warr(w):                                # [1024, n] -> [128, 8*n] chunk-major
        n = w.shape[1]
        return np.ascontiguousarray(
            w.reshape(8, 128, n).transpose(1, 0, 2).reshape(128, 8 * n).astype(bf)
        )

    wp_full = warr(w_proj)                      # [128, 8*1024] bf16
    bp = np.ascontiguousarray(b_proj.reshape(8, 128).T)

    in_maps = []
    for c in range(NCORES):
        wq_c = w_qkv[:, 0 * C + c * PC : 0 * C + (c + 1) * PC] @ mq
        wk_c = w_qkv[:, 1 * C + c * PC : 1 * C + (c + 1) * PC]
        wv_c = w_qkv[:, 2 * C + c * PC : 2 * C + (c + 1) * PC] @ mv
        bq_c = b_qkv[0 * C + c * PC : 0 * C + (c + 1) * PC] @ mq
        bk_c = b_qkv[1 * C + c * PC : 1 * C + (c + 1) * PC]
        bv_c = b_qkv[2 * C + c * PC : 2 * C + (c + 1) * PC] @ mv
        in_maps.append(
            {
                "xt": xt,
                "wq": warr(wq_c),
                "wk": warr(wk_c),
                "wv": warr(wv_c),
                "bq": np.ascontiguousarray(bq_c.reshape(128, 1)),
                "bk": np.ascontiguousarray(bk_c.reshape(128, 1)),
                "bv": np.ascontiguousarray(bv_c.reshape(128, 1)),
                "wp": wp_full,
                "bp": bp,
                "eye64x2": eye64x2,
            }
        )
    return in_maps


def run_sharded(inputs, trace=False, **kw):
    nc = _get_nc()
    in_maps = _make_in_maps(inputs)
    res = run_bass_kernel_spmd(nc, in_maps, list(range(NCORES)), trace=trace, **kw)
    # results[c]["out"]: [B, C, TPC] -- core c's token shard of final y^T
    yT = np.concatenate([res.results[c]["out"] for c in range(NCORES)], axis=2)
    out = np.ascontiguousarray(yT.transpose(0, 2, 1))  # [B, N, C]
    return out, res


def kernel(**inputs) -> np.ndarray:
    out, _ = run_sharded(inputs, trace=False)
    return out


# revision 28
# speedup vs baseline: 1.0243x; 1.0243x over previous
"""Multi-head attention with q/v LoRA on 8 trn2 NeuronCores (bf16 PE path).

Reference computation (B=2, N=2048, C=1024, H=16, HD=64, R=16):
    qkv = x @ w_qkv + b_qkv                -> split per-head q, k, v
    q  += ((q @ a_q) @ b_q) * 2.0          (per head; same for v)
    out = softmax(q k^T / 8) v             (full N x N scores)
    y   = out @ w_proj + b_proj

Sharding: tensor-parallel over heads -- each of the 8 cores owns 2 heads
(128 of the 3*1024 qkv columns) for both batches; attention output is
resharded over tokens with a per-(batch,head) AllToAll so each core
computes final proj rows for its 256 tokens per batch with the full
w_proj.

Key implementation choices:
  * All PE operands are bf16 (hardware runs fp32r at ~2 cycles/row; bf16
    at 1).  PSUM accumulation stays fp32, biases stay fp32.
  * The LoRA is linear in q/v, so it is folded into the qkv weights on
    the host: w_eff = w @ (I + blockdiag(a@b)*scale), same for bias.
    Nothing LoRA-related runs on device.
  * x^T, weights are pre-cast to bf16 on the host and DMA'd straight
    into matmul operand tiles (no on-device rounding copies).
  * Softmax: scores S^T = k^T' q^T -> exp on ACT (bf16 out) -> P @ [v|1]
    in PSUM (ones column gives the row sums).  1/sums via the fast DVE
    reciprocal, broadcast to 64 partitions on the idle GpSimd engine,
    one fused multiply writes normalized bf16 O^T for the AllToAll.
  * v_aug ones columns are memset once into two persistent buffers.
  * AllToAll payloads are bf16 (256 KB per (batch, head)); the receive
    side DMAs the collective output straight into the proj operand tile.
The host stitches the 8 token shards and transposes back to [B, N, C].
"""

import sys

sys.path.insert(0, "/opt/trn_rl_repo")
sys.path.insert(0, "/root/.axon_site")

import numpy as np
import ml_dtypes

import concourse.bass as bass
import concourse.mybir as mybir
import concourse.tile as tile
from concourse.bass_utils import run_bass_kernel_spmd

f32 = mybir.dt.float32
bf16 = mybir.dt.bfloat16
AF = mybir.ActivationFunctionType

B, N, C = 2, 2048, 1024
H, HD, R = 16, 64, 16
LORA_SCALE = 32.0 / R
ATTN_SCALE = HD ** -0.5
NCORES = 8
HPC = H // NCORES          # heads per core = 2
PC = HPC * HD              # partition columns per core = 128
ROWS = B * N               # 4096 tokens
RC = 512                   # row-chunk size for qkv production
TPC = N // NCORES          # tokens per core per batch = 256


def _legalize_waits(nc, max_waits=1):
    """This walrus build accepts at most one sync-wait per instruction;
    Tile attaches several.  Move surplus waits onto same-engine NoOps
    inserted immediately before the instruction (identical semantics)."""
    counter = 0
    for fn in nc.m.functions:
        for bb in fn.blocks:
            insts = bb.instructions
            out = []
            changed = False
            for inst in insts:
                si = inst.sync_info
                if si is not None and si.on_wait and len(si.on_wait) > max_waits:
                    waits = list(si.on_wait)
                    for w in waits[:-max_waits]:
                        counter += 1
                        nop = mybir.InstNoOp(
                            name=f"I-wfix-{counter}",
                            engine=inst.engine,
                            sync_info=mybir.SyncInfo(on_wait=[w], on_update=[]),
                        )
                        nc.register_instruction(nop)
                        out.append(nop)
                    si.on_wait.clear()
                    si.on_wait.extend(waits[-max_waits:])
                    changed = True
                out.append(inst)
            if changed:
                insts[:] = out


def build_nc():
    nc = bass.Bass(num_devices=NCORES)

    xt_d = nc.dram_tensor("xt", [C, ROWS], bf16, kind="ExternalInput")
    wq_d = nc.dram_tensor("wq", [128, 1024], bf16, kind="ExternalInput")
    wk_d = nc.dram_tensor("wk", [128, 1024], bf16, kind="ExternalInput")
    wv_d = nc.dram_tensor("wv", [128, 1024], bf16, kind="ExternalInput")
    bq_d = nc.dram_tensor("bq", [128, 1], f32, kind="ExternalInput")
    bk_d = nc.dram_tensor("bk", [128, 1], f32, kind="ExternalInput")
    bv_d = nc.dram_tensor("bv", [128, 1], f32, kind="ExternalInput")
    wp_d = nc.dram_tensor("wp", [128, 8 * 1024], bf16, kind="ExternalInput")
    bp_d = nc.dram_tensor("bp", [128, 8], f32, kind="ExternalInput")
    eye64x2_d = nc.dram_tensor("eye64x2", [128, 64], bf16, kind="ExternalInput")
    out_d = nc.dram_tensor("out", [B, C, TPC], f32, kind="ExternalOutput")

    with nc.allow_low_precision(
        reason="bf16 matmul operands are intended; PSUM accumulation stays fp32"
    ), tile.TileContext(nc) as tc:
        with (
            tc.tile_pool(name="persist", bufs=1) as persist,
            tc.tile_pool(name="const", bufs=1) as const,
            tc.tile_pool(name="dram", bufs=1, space="DRAM") as dram,
            tc.tile_pool(name="xio", bufs=2) as xio_p,
            tc.tile_pool(name="work", bufs=2) as work_p,
            tc.tile_pool(name="ps", bufs=1, space="PSUM") as ps,
        ):
            qT = persist.tile([128, ROWS], bf16, tag="qT", name="qT")
            kT = persist.tile([128, ROWS], bf16, tag="kT", name="kT")
            vT = persist.tile([128, ROWS], bf16, tag="vT", name="vT")

            # prefetch the first x^T chunk ahead of the weight DMAs
            xT0 = xio_p.tile([128, 8 * RC], bf16, tag="xT", bufs=6, name="xT00")
            nc.sync.dma_start(
                out=xT0[:].rearrange("p (a r) -> p a r", a=8),
                in_=xt_d[:, 0:RC].rearrange("(a p) r -> p a r", p=128),
            )

            w_t = []
            for nm, d in (("wq", wq_d), ("wk", wk_d), ("wv", wv_d)):
                t = const.tile([128, 1024], bf16, tag=nm, name=f"{nm}_t")
                nc.sync.dma_start(out=t[:], in_=d[:])
                w_t.append(t)

            eye64x2 = const.tile([128, 64], bf16, tag="eye64", name="eye64")
            nc.sync.dma_start(out=eye64x2[:], in_=eye64x2_d[:])

            bias_t = []
            for nm, d in (("bq", bq_d), ("bk", bk_d), ("bv", bv_d)):
                bt = const.tile([128, 1], f32, tag=nm, name=f"{nm}_t")
                nc.sync.dma_start(out=bt[:], in_=d[:])
                bias_t.append(bt)
            bp_t = const.tile([128, 8], f32, tag="bp", name="bp_t")
            nc.sync.dma_start(out=bp_t[:], in_=bp_d[:])

            wp_t = const.tile([128, 8 * 1024], bf16, tag="wp_t", name="wp_t")

            # persistent v_aug buffers (one per (batch, head)): ones columns
            # written once by memset, data blocks overwritten by transposes
            v_aug_bufs = {}
            for b in range(B):
                for hl in range(HPC):
                    va = persist.tile(
                        [128, 16 * 65], bf16, tag=f"va{b}{hl}", name=f"va{b}{hl}"
                    )
                    nc.gpsimd.memset(va[:], 1.0)
                    v_aug_bufs[(b, hl)] = va

            ones_row = const.tile([1, 64], bf16, tag="ones_r", name="ones_r")
            nc.gpsimd.memset(ones_row[:], 1.0)

            qkvT = (qT, kT, vT)

            xts = {}

            def load_xchunk(b, rci):
                r0 = b * N + rci * RC
                xT_t = xio_p.tile(
                    [128, 8 * RC], bf16, tag="xT", bufs=6, name=f"xT{b}{rci}"
                )
                nc.sync.dma_start(
                    out=xT_t[:].rearrange("p (a r) -> p a r", a=8),
                    in_=xt_d[:, r0 : r0 + RC].rearrange("(a p) r -> p a r", p=128),
                )
                xts[(b, rci)] = xT_t
                return xT_t

            def emit_qkv_m(b, rci, m, act_ok=True):
                r0 = b * N + rci * RC
                xT_t = xts[(b, rci)]
                acc = ps.tile([128, RC], f32, tag="acc", bufs=2, name=f"ac{b}{rci}{m}")
                for ci in range(8):
                    nc.tensor.matmul(
                        acc[:],
                        w_t[m][:, ci * 128 : (ci + 1) * 128],
                        xT_t[:, ci * RC : (ci + 1) * RC],
                        start=(ci == 0),
                        stop=(ci == 7),
                    )
                dst = qkvT[m][:, r0 : r0 + RC]
                if m == 0 and act_ok:
                    nc.scalar.activation(dst, acc[:], AF.Identity, bias=bias_t[m][:])
                else:
                    nc.vector.tensor_scalar_add(dst, acc[:], bias_t[m][:])

            def emit_qkv_chunk(b, rci, xT_t=None, act_ok=True):
                if (b, rci) not in xts:
                    if xT_t is not None:
                        xts[(b, rci)] = xT_t
                    else:
                        load_xchunk(b, rci)
                for m in range(3):
                    emit_qkv_m(b, rci, m, act_ok=act_ok)

            def emit_vaug(b, hl):
                boff = b * N
                hs = slice(hl * HD, (hl + 1) * HD)
                v_aug = v_aug_bufs[(b, hl)]
                for kt4 in range(4):
                    vtr = ps.tile([128, 256], bf16, tag="s", bufs=2, name=f"vt{b}{hl}{kt4}")
                    for j in range(4):
                        ko = boff + (kt4 * 4 + j) * 128
                        nc.tensor.transpose(
                            vtr[:, j * 64 : (j + 1) * 64],
                            vT[hs, ko : ko + 128],
                            eye64x2[hs, :],
                        )
                    nc.vector.tensor_copy(
                        v_aug[:].rearrange("p (k e) -> p k e", e=65)[
                            :, kt4 * 4 : kt4 * 4 + 4, 0:64
                        ],
                        vtr[:].rearrange("p (k e) -> p k e", e=64),
                    )
                return v_aug

            # filler queue: small batches of independent PE work emitted
            # between attention kt iterations so the tensor engine never
            # starves while waiting on the ACT exp cadence
            filler_q = []

            def pop_filler():
                if filler_q:
                    filler_q.pop(0)()

            def emit_unit(b, hl, qh, a2a_in, pops=(4, 8, 12)):
                """Emit scores/exp/PV for one (batch, head, q-half) unit.
                Returns a finisher closure (normalize + a2a staging DMA) to
                be emitted later -- after the next unit's first matmuls -- so
                the slow reciprocal never blocks the in-order PE queue."""
                boff = b * N
                hs = slice(hl * HD, (hl + 1) * HD)
                qoff = boff + qh * 1024
                v_aug = v_aug_bufs[(b, hl)]
                o_ps = ps.tile([65, 1024], f32, tag="o", bufs=1, name=f"o{b}{hl}{qh}")

                def emit_pv(p_tile, kt):
                    for qc in range(2):
                        nc.tensor.matmul(
                            o_ps[:, qc * 512 : (qc + 1) * 512],
                            v_aug[:, kt * 65 : kt * 65 + 65],
                            p_tile[:, qc * 512 : (qc + 1) * 512],
                            start=(kt == 0),
                            stop=(kt == 15),
                        )

                pending = None
                for kt in range(16):
                    ko = boff + kt * 128
                    s_ps = ps.tile([128, 1024], f32, tag="s", bufs=2, name=f"s{b}{hl}{qh}{kt}")
                    for qc in range(2):
                        nc.tensor.matmul(
                            s_ps[:, qc * 512 : (qc + 1) * 512],
                            kT[hs, ko : ko + 128],
                            qT[hs, qoff + qc * 512 : qoff + (qc + 1) * 512],
                            start=True,
                            stop=True,
                        )
                    p_sb = work_p.tile([128, 1024], bf16, tag="p", bufs=3, name=f"p{qh}{kt}")
                    nc.scalar.activation(p_sb[:], s_ps[:], AF.Exp, scale=ATTN_SCALE)
                    if pending is not None:
                        emit_pv(*pending)
                        if kt in pops:
                            pop_filler()
                    pending = (p_sb, kt)
                emit_pv(*pending)
                # DVE-side epilogue now (doesn't touch the PE queue): copy
                # O^T + sums out of PSUM, take reciprocals per q-half
                nst = work_p.tile([65, 1024], f32, tag="nst", bufs=2, name=f"n{hl}{qh}")
                nc.vector.tensor_copy(nst[:], o_ps[:])
                r_bf = work_p.tile([1, 1024], bf16, tag="rbf", bufs=2, name=f"rb{b}{hl}{qh}")
                for rq in range(4):
                    r_sb = work_p.tile([1, 256], f32, tag="r", bufs=2, name=f"r{b}{hl}{qh}{rq}")
                    nc.vector.reciprocal(r_sb[:], nst[64:65, rq * 256 : (rq + 1) * 256])
                    nc.vector.tensor_copy(r_bf[:, rq * 256 : (rq + 1) * 256], r_sb[:])

                def finisher():
                    onrm = work_p.tile(
                        [64, 1024], bf16, tag="onrm", bufs=2, name=f"on{b}{hl}{qh}"
                    )
                    for qc in range(2):
                        bc_ps = ps.tile([64, 512], f32, tag="acc", bufs=2, name=f"bc{qc}")
                        nc.tensor.matmul(
                            bc_ps[:],
                            ones_row[:],
                            r_bf[:, qc * 512 : (qc + 1) * 512],
                            start=True,
                            stop=True,
                        )
                        nc.vector.tensor_mul(
                            onrm[:, qc * 512 : (qc + 1) * 512],
                            nst[0:64, qc * 512 : (qc + 1) * 512],
                            bc_ps[:],
                        )
                    nc.sync.dma_start(
                        out=a2a_in[qh * 4 : (qh + 1) * 4, :, :].rearrange(
                            "a p r -> p a r"
                        ),
                        in_=onrm[:].rearrange("p (a r) -> p a r", a=4),
                    )

                return finisher

            def emit_a2a(b, hl, a2a_in):
                a2a_out = dram.tile(
                    [8, 64, TPC], bf16, tag=f"ao{b}{hl}", name=f"ao{b}{hl}"
                )
                nc.gpsimd.collective_compute(
                    "AllToAll",
                    mybir.AluOpType.bypass,
                    replica_groups=[list(range(NCORES))],
                    ins=[a2a_in[:].opt()],
                    outs=[a2a_out[:].opt()],
                )
                return a2a_out

            def new_a2a_in(b, hl):
                return dram.tile([8, 64, TPC], bf16, tag=f"ai{b}{hl}", name=f"ai{b}{hl}")

            recv_tiles = {}

            def get_recv(b):
                if b not in recv_tiles:
                    recv_tiles[b] = work_p.tile(
                        [128, 8 * TPC], bf16, tag=f"rcr{b}", bufs=1, name=f"rr{b}"
                    )
                return recv_tiles[b]

            def emit_recv_head(b, hl, a2a_out):
                recv_r = get_recv(b)
                nc.sync.dma_start(
                    out=recv_r[hl * 64 : (hl + 1) * 64, :].rearrange(
                        "p (a r) -> p a r", a=8
                    ),
                    in_=a2a_out[:].rearrange("a p r -> p a r"),
                )
                return recv_r

            def emit_proj_mt(b, recv_r, mt):
                y_ps = ps.tile([128, TPC], f32, tag="acc", bufs=2, name=f"y{b}{mt}")
                for kc in range(8):
                    nc.tensor.matmul(
                        y_ps[:],
                        wp_t[:, kc * 1024 + mt * 128 : kc * 1024 + (mt + 1) * 128],
                        recv_r[:, kc * TPC : (kc + 1) * TPC],
                        start=(kc == 0),
                        stop=(kc == 7),
                    )
                yst = work_p.tile([128, TPC], f32, tag="yst", bufs=3, name=f"ys{b}{mt}")
                nc.vector.tensor_scalar_add(yst[:], y_ps[:], bp_t[:, mt : mt + 1])
                nc.sync.dma_start(
                    out=out_d[b, mt * 128 : (mt + 1) * 128, :], in_=yst[:]
                )

            def emit_proj_pass(b, recv_r, mt, y_ap, hl, last):
                """K=64 proj pass over one head-pair block; accumulates into
                y_ap across the two passes (hl=0 start, hl=1 stop)."""
                lo, hi = hl * 64, (hl + 1) * 64
                for kc in range(8):
                    nc.tensor.matmul(
                        y_ap,
                        wp_t[lo:hi, kc * 1024 + mt * 128 : kc * 1024 + (mt + 1) * 128],
                        recv_r[lo:hi, kc * TPC : (kc + 1) * TPC],
                        start=(hl == 0 and kc == 0),
                        stop=(last and kc == 7),
                        skip_group_check=True,
                    )
                if last:
                    yst = work_p.tile(
                        [128, TPC], f32, tag="yst", bufs=3, name=f"ys{b}{mt}"
                    )
                    nc.vector.tensor_scalar_add(yst[:], y_ap, bp_t[:, mt : mt + 1])
                    nc.sync.dma_start(
                        out=out_d[b, mt * 128 : (mt + 1) * 128, :], in_=yst[:]
                    )

            # ---- emission schedule ---------------------------------------
            # Phase 1: qkv(b0) back-to-back (dense PE stream ramps the
            # clock) + b0 v_aug transposes; b1 x-chunk DMAs pre-issued.
            emit_qkv_chunk(0, 0, xT_t=xT0)
            for rci in range(1, 4):
                emit_qkv_chunk(0, rci)
            for rci in range(4):
                load_xchunk(1, rci)
            nc.sync.dma_start(out=wp_t[:], in_=wp_d[:])
            emit_vaug(0, 0)
            emit_vaug(0, 1)

            # Phase 2: attention units.  qkv(b1), v_aug(b1), normalize
            # finishers, a2a/recv issues and proj blocks all run as filler
            # between kt iterations so the tensor engine never idles (idle
            # gaps drop the PE out of its boosted clock state).
            ai = {
                (b, hl): new_a2a_in(b, hl) for b in range(B) for hl in range(HPC)
            }
            ao = {}
            recv0 = get_recv(0)
            recv1 = get_recv(1)

            for rci in range(4):
                for m in range(3):
                    filler_q.append(
                        lambda rci=rci, m=m: emit_qkv_m(1, rci, m, act_ok=False)
                    )
            filler_q.append(lambda: emit_vaug(1, 0))
            filler_q.append(lambda: emit_vaug(1, 1))

            fin = emit_unit(0, 0, 0, ai[(0, 0)])
            filler_q.insert(0, fin)
            fin = emit_unit(0, 0, 1, ai[(0, 0)])
            filler_q.insert(0, fin)
            filler_q.insert(1, lambda: ao.update({(0, 0): emit_a2a(0, 0, ai[(0, 0)])}))
            fin = emit_unit(0, 1, 0, ai[(0, 1)])
            filler_q.insert(0, fin)
            fin = emit_unit(0, 1, 1, ai[(0, 1)])
            filler_q.insert(0, fin)
            filler_q.insert(1, lambda: ao.update({(0, 1): emit_a2a(0, 1, ai[(0, 1)])}))
            filler_q.insert(2, lambda: emit_recv_head(0, 0, ao[(0, 0)]))
            # b1 attention consumes qkv(b1)/v_aug(b1): drain any fillers
            # that haven't been popped yet before the first b1 unit
            while filler_q:
                pop_filler()
            fin = emit_unit(1, 0, 0, ai[(1, 0)])
            filler_q.insert(0, fin)
            filler_q.insert(1, lambda: emit_recv_head(0, 1, ao[(0, 1)]))
            for mt in range(4):
                filler_q.append(lambda mt=mt: emit_proj_mt(0, recv0, mt))
            fin = emit_unit(1, 0, 1, ai[(1, 0)])
            filler_q.insert(0, fin)
            filler_q.insert(1, lambda: ao.update({(1, 0): emit_a2a(1, 0, ai[(1, 0)])}))
            fin = emit_unit(1, 1, 0, ai[(1, 1)])
            filler_q.insert(0, fin)
            filler_q.insert(1, lambda: emit_recv_head(1, 0, ao[(1, 0)]))
            for mt in range(4, 8):
                filler_q.append(lambda mt=mt: emit_proj_mt(0, recv0, mt))
            fin = emit_unit(1, 1, 1, ai[(1, 1)])
            while filler_q:
                pop_filler()
            fin()
            ao[(1, 1)] = emit_a2a(1, 1, ai[(1, 1)])

            # Phase 3: proj(1) in two K=64 passes -- pass A (head block 0)
            # overlaps the in-flight a2a(1,1); pass B after its receive.
            # PSUM matmul start=True resets the whole 2 KB bank, so every mt
            # accumulator must own a distinct bank: spread the 8 groups over
            # the (now idle) s/o/acc pool slots, one group per bank.
            y1a = ps.tile([128, 1024], f32, tag="s", bufs=2, name="y1a")
            y1b = ps.tile([128, 1024], f32, tag="s", bufs=2, name="y1b")
            y1c = ps.tile([128, 1024], f32, tag="o", bufs=1, name="y1c")
            y1d = ps.tile([128, 512], f32, tag="acc", bufs=2, name="y1d")
            y1e = ps.tile([128, 512], f32, tag="acc", bufs=2, name="y1e")

            def y_ap(mt):
                # mts 0-5: two per two-bank tile, one per bank (cols 0-255
                # in bank 0, cols 512-767 in bank 1); mts 6-7: one-bank tiles
                if mt < 6:
                    t = (y1a, y1b, y1c)[mt // 2]
                    return t[:, (mt % 2) * 512 : (mt % 2) * 512 + TPC]
                t = y1d if mt == 6 else y1e
                return t[:, 0:TPC]

            for mt in range(8):
                emit_proj_pass(1, recv1, mt, y_ap(mt), 0, last=False)
            emit_recv_head(1, 1, ao[(1, 1)])
            for mt in range(8):
                emit_proj_pass(1, recv1, mt, y_ap(mt), 1, last=True)

    _legalize_waits(nc)
    return nc


_NC_CACHE = None


def _get_nc():
    global _NC_CACHE
    if _NC_CACHE is None:
        _NC_CACHE = build_nc()
    return _NC_CACHE


def _make_in_maps(inputs):
    bf = ml_dtypes.bfloat16
    x = np.ascontiguousarray(np.asarray(inputs["x"], dtype=np.float32)).reshape(ROWS, C)
    xt = np.ascontiguousarray(x.T.astype(bf))   # [C, ROWS] bf16
    w_qkv = np.asarray(inputs["w_qkv"], dtype=np.float32)
    b_qkv = np.asarray(inputs["b_qkv"], dtype=np.float32)
    a_q = np.asarray(inputs["a_q"], dtype=np.float32)
    b_q = np.asarray(inputs["b_q"], dtype=np.float32)
    a_v = np.asarray(inputs["a_v"], dtype=np.float32)
    b_v = np.asarray(inputs["b_v"], dtype=np.float32)
    w_proj = np.asarray(inputs["w_proj"], dtype=np.float32)
    b_proj = np.asarray(inputs["b_proj"], dtype=np.float32)

    # fold the (linear) per-head LoRA into the q/v weights and biases:
    # q_final = (x@w_q + b_q) @ (I + a_q@b_q * scale)
    dq = a_q @ b_q * LORA_SCALE                 # [64, 64]
    dv = a_v @ b_v * LORA_SCALE
    mq = np.eye(128, dtype=np.float32)
    mq[0:64, 0:64] += dq
    mq[64:128, 64:128] += dq
    mv = np.eye(128, dtype=np.float32)
    mv[0:64, 0:64] += dv
    mv[64:128, 64:128] += dv

    eye64x2 = np.vstack([np.eye(64, dtype=np.float32)] * 2).astype(bf)

    def warr(w):                                # [1024, n] -> [128, 8*n] chunk-major
        n = w.shape[1]
        return np.ascontiguousarray(
            w.reshape(8, 128, n).transpose(1, 0, 2).reshape(128, 8 * n).astype(bf)
        )

    wp_full = warr(w_proj)                      # [128, 8*1024] bf16
    bp = np.ascontiguousarray(b_proj.reshape(8, 128).T)

    in_maps = []
    for c in range(NCORES):
        wq_c = w_qkv[:, 0 * C + c * PC : 0 * C + (c + 1) * PC] @ mq
        wk_c = w_qkv[:, 1 * C + c * PC : 1 * C + (c + 1) * PC]
        wv_c = w_qkv[:, 2 * C + c * PC : 2 * C + (c + 1) * PC] @ mv
        bq_c = b_qkv[0 * C + c * PC : 0 * C + (c + 1) * PC] @ mq
        bk_c = b_qkv[1 * C + c * PC : 1 * C + (c + 1) * PC]
        bv_c = b_qkv[2 * C + c * PC : 2 * C + (c + 1) * PC] @ mv
        in_maps.append(
            {
                "xt": xt,
                "wq": warr(wq_c),
                "wk": warr(wk_c),
                "wv": warr(wv_c),
                "bq": np.ascontiguousarray(bq_c.reshape(128, 1)),
                "bk": np.ascontiguousarray(bk_c.reshape(128, 1)),
                "bv": np.ascontiguousarray(bv_c.reshape(128, 1)),
                "wp": wp_full,
                "bp": bp,
                "eye64x2": eye64x2,
            }
        )
    return in_maps


def run_sharded(inputs, trace=False, **kw):
    nc = _get_nc()
    in_maps = _make_in_maps(inputs)
    res = run_bass_kernel_spmd(nc, in_maps, list(range(NCORES)), trace=trace, **kw)
    # results[c]["out"]: [B, C, TPC] -- core c's token shard of final y^T
    yT = np.concatenate([res.results[c]["out"] for c in range(NCORES)], axis=2)
    out = np.ascontiguousarray(yT.transpose(0, 2, 1))  # [B, N, C]
    return out, res


def kernel(**inputs) -> np.ndarray:
    out, _ = run_sharded(inputs, trace=False)
    return out
